# revision 1
# baseline (speedup 1.0000x reference)
"""InterpretableMultimodalCapsuleFusion — hand-written Bass/Tile kernel for TRN2.

Contract: kernel(**inputs) takes FULL unsharded inputs (numpy), returns FULL
output [B, 1] float32.  Strategy: pure data parallel over 8 NeuronCores
(batch shard of 128/core, weights replicated), one SPMD Bass program via
run_bass_kernel_spmd.

Per-core program layout (B=128 batch on the free axis everywhere):

Encoders (3 modalities x bidirectional LSTM, H=64):
  - gates live in PSUM as [128 part = (fwd 0:64 | bwd 64:128), 512 = I|F|O|G]
    per modality (1 bank), pair-interleaved so every elementwise op uses all
    128 partitions.
  - input projections (px) are folded directly into the PSUM accumulation:
    col-tiled M=64 matmuls (tile_position (0,0)/(0,64)) from a feature-major
    bf16 copy of the inputs (host pre-transposed, ones-row carries the bias).
  - recurrent term: one block-diagonal [128,128] bf16 matmul per gate
    accumulating on top (start=False).
  - elementwise split in 2 groups (text+audio / video) to overlap the serial
    dependency chain with ACT/DVE throughput.

Routing (4 iterations): gates-on-partition layout, sigmoid-trick softmax for
the n=2 coefficient tensors, exp + DVE reciprocal for n=3/7, agreement dots
via an all-ones stationary matmul (result arrives partition-broadcast).

Falls back to a pure numpy reference implementation on any failure.
"""

import os
import sys
import traceback

import numpy as np

for _p in ("/opt/trn_rl_repo",):
    if os.path.isdir(_p) and _p not in sys.path:
        sys.path.insert(0, _p)

B, T = 1024, 128
T_IN, A_IN, V_IN = 300, 74, 35
D = 128
H = D // 2
ROUTING = 3
N_CORES = 8
BC = B // N_CORES  # batch per core

_WEIGHT_KEYS = [
    "t_Wih_f", "t_Whh_f", "t_b_f", "t_Wih_b", "t_Whh_b", "t_b_b",
    "a_Wih_f", "a_Whh_f", "a_b_f", "a_Wih_b", "a_Whh_b", "a_b_b",
    "v_Wih_f", "v_Whh_f", "v_b_f", "v_Wih_b", "v_Whh_b", "v_b_b",
    "Wt", "Wa", "Wv", "r_Wih", "r_Whh", "r_b",
    "d_Wih_f", "d_Whh_f", "d_b_f", "d_Wih_b", "d_Whh_b", "d_b_b",
    "fc1_W", "fc1_b", "fc2_W", "fc2_b",
]

# torch gate row order is i, f, g, o; we lay gates out as I, F, O, G.
def _gate_slices(h):
    return [(0, h), (h, 2 * h), (3 * h, 4 * h), (2 * h, 3 * h)]  # I F O G

MODS = ["t", "a", "v"]
FEATS = [T_IN, A_IN, V_IN]
KROWS = [T_IN + 1, A_IN + 1, V_IN + 1]  # + ones row for the bias
K_CHUNKS = [
    [(0, 128), (128, 256), (256, 301)],
    [(0, 75)],
    [(0, 36)],
]
WIN = 8  # encoder x-window, steps per DMA

# routing pair chains: tokens as (modality, k) indices into usc
PAIR_TOKENS = [
    [(0, 0), (1, 0)],           # ta
    [(0, 1), (2, 0)],           # tv
    [(1, 1), (2, 1)],           # av
    [(0, 2), (1, 2), (2, 2)],   # tav
]
DECI_USC_TOKENS = [(0, 3), (1, 3), (2, 3)]
PAIR_N = [2, 2, 2, 3]
# rc layout along free dim (blocks of 128): ta(2) tv(2) av(2) tav(3) deci(7)
RC_OFF = [0, 256, 512, 768]
RC_DECI = 1152
RC_TOT = 2048


# ---------------------------------------------------------------------------
# host-side weight/input packing
# ---------------------------------------------------------------------------

def _pack_weights(w, bf16):
    """Build all device weight tensors (bf16) from the raw inputs."""
    out = {}
    gsH = _gate_slices(H)
    gsD = _gate_slices(D)

    for mi, m in enumerate(MODS):
        K = KROWS[mi]
        wpx = np.zeros((K, 512), np.float32)
        for gi, (r0, r1) in enumerate(gsH):
            wpx[: K - 1, gi * 128 : gi * 128 + 64] = w[m + "_Wih_f"][r0:r1].T
            wpx[: K - 1, gi * 128 + 64 : gi * 128 + 128] = w[m + "_Wih_b"][r0:r1].T
            wpx[K - 1, gi * 128 : gi * 128 + 64] = w[m + "_b_f"][r0:r1]
            wpx[K - 1, gi * 128 + 64 : gi * 128 + 128] = w[m + "_b_b"][r0:r1]
        out["wpx_" + m] = wpx.astype(bf16)

    wrec = np.zeros((128, 12 * 128), np.float32)
    for mi, m in enumerate(MODS):
        for gi, (r0, r1) in enumerate(gsH):
            blk = wrec[:, (mi * 4 + gi) * 128 : (mi * 4 + gi + 1) * 128]
            blk[0:64, 0:64] = w[m + "_Whh_f"][r0:r1].T
            blk[64:128, 64:128] = w[m + "_Whh_b"][r0:r1].T
    out["wrec"] = wrec.astype(bf16)

    wcap = np.zeros((128, 12 * 128), np.float32)
    for mi, wk in enumerate(["Wt", "Wa", "Wv"]):
        for k in range(4):
            wcap[:, (mi * 4 + k) * 128 : (mi * 4 + k + 1) * 128] = w[wk][k]
    out["wcap"] = wcap.astype(bf16)

    wr_ih = np.zeros((128, 4 * 512), np.float32)
    wr_hh = np.zeros((128, 4 * 512), np.float32)
    rb = np.zeros((1, 4 * 512), np.float32)
    for i in range(4):
        for gi, (r0, r1) in enumerate(gsD):
            wr_ih[:, i * 512 + gi * 128 : i * 512 + (gi + 1) * 128] = \
                w["r_Wih"][i][r0:r1].T
            wr_hh[:, i * 512 + gi * 128 : i * 512 + (gi + 1) * 128] = \
                w["r_Whh"][i][r0:r1].T
            rb[0, i * 512 + gi * 128 : i * 512 + (gi + 1) * 128] = \
                w["r_b"][i][r0:r1]
    out["wr_ih"] = wr_ih.astype(bf16)
    out["wr_hh"] = wr_hh.astype(bf16)
    out["rb"] = rb.astype(bf16)

    wd_ih = np.zeros((128, 2 * 512), np.float32)
    wd_hh = np.zeros((128, 2 * 512), np.float32)
    db = np.zeros((1, 2 * 512), np.float32)
    for di, d in enumerate(["f", "b"]):
        for gi, (r0, r1) in enumerate(gsD):
            wd_ih[:, di * 512 + gi * 128 : di * 512 + (gi + 1) * 128] = \
                w["d_Wih_" + d][r0:r1].T
            wd_hh[:, di * 512 + gi * 128 : di * 512 + (gi + 1) * 128] = \
                w["d_Whh_" + d][r0:r1].T
            db[0, di * 512 + gi * 128 : di * 512 + (gi + 1) * 128] = \
                w["d_b_" + d][r0:r1]
    out["wd_ih"] = wd_ih.astype(bf16)
    out["wd_hh"] = wd_hh.astype(bf16)
    out["db"] = db.astype(bf16)

    out["fc1wt"] = np.ascontiguousarray(w["fc1_W"].T).astype(bf16)  # [128, 64]
    out["fc2wt"] = np.ascontiguousarray(w["fc2_W"].T).astype(bf16)  # [64, 1]
    out["fc1b"] = w["fc1_b"].reshape(H, 1).astype(np.float32)
    out["fc2b"] = w["fc2_b"].reshape(1, 1).astype(np.float32)
    out["ones"] = np.ones((128, 128), np.float32).astype(bf16)
    out["zeros"] = np.zeros((128, 384), np.float32).astype(bf16)
    rc0 = np.zeros((128, RC_TOT), np.float32)
    for i in range(4):
        rc0[:, RC_OFF[i]:RC_OFF[i] + PAIR_N[i] * 128] = 1.0 / PAIR_N[i]
    rc0[:, RC_DECI:RC_TOT] = 1.0 / 7.0
    out["rc0"] = rc0.astype(bf16)
    return out


def _pack_x(x, feat, bf16):
    """[B, T, feat] fp32 -> per-core feature-major bf16 with ones row:
    [N_CORES, feat+1, T*BC] with column index t*BC + b."""
    xc = x.reshape(N_CORES, BC, T, feat).transpose(0, 3, 2, 1)  # [8, F, T, BC]
    xc = np.ascontiguousarray(xc).reshape(N_CORES, feat, T * BC)
    outp = np.empty((N_CORES, feat + 1, T * BC), bf16)
    outp[:, :feat] = xc.astype(bf16)
    outp[:, feat] = np.float32(1.0)
    return outp


# ---------------------------------------------------------------------------
# device program
# ---------------------------------------------------------------------------

_DEBUG = False  # when True, the program also dumps intermediates
_ENC_ONLY = False   # timing: stop after encoder+capsule
_SKIP_ENC = False   # timing: memset usc, routing only
_NO_REC = False     # timing probe: drop recurrent matmuls (wrong numerics)


def _build_program(has_rbias, has_dbias):
    import concourse.mybir as mybir
    from concourse import bacc
    from concourse.tile import TileContext
    from concourse.alu_op_type import AluOpType

    f32 = mybir.dt.float32
    bf16 = mybir.dt.bfloat16
    AF = mybir.ActivationFunctionType
    MUL, ADD, SUB = AluOpType.mult, AluOpType.add, AluOpType.subtract

    nc = bacc.Bacc()

    # ---- DRAM parameters -------------------------------------------------
    xin_d = {}
    for mi, m in enumerate(MODS):
        for ci, (r0, r1) in enumerate(K_CHUNKS[mi]):
            xin_d[(mi, ci)] = nc.declare_dram_parameter(
                f"x_{m}{ci}", [r1 - r0, T * BC], bf16, isOutput=False)
    wp = {}
    for name, shape, dt in [
        ("wpx_t", [KROWS[0], 512], bf16), ("wpx_a", [KROWS[1], 512], bf16),
        ("wpx_v", [KROWS[2], 512], bf16), ("wrec", [128, 1536], bf16),
        ("wcap", [128, 1536], bf16),
        ("wr_ih", [128, 2048], bf16), ("wr_hh", [128, 2048], bf16),
        ("wd_ih", [128, 1024], bf16), ("wd_hh", [128, 1024], bf16),
        ("rb", [1, 2048], bf16), ("db", [1, 1024], bf16),
        ("fc1wt", [128, H], bf16), ("fc2wt", [H, 1], bf16),
        ("fc1b", [H, 1], f32), ("fc2b", [1, 1], f32),
        ("ones", [128, 128], bf16), ("zeros", [128, 384], bf16),
        ("rc0", [128, RC_TOT], bf16),
    ]:
        wp[name] = nc.declare_dram_parameter(name, shape, dt, isOutput=False)
    out_d = nc.declare_dram_parameter("out", [1, BC], f32, isOutput=True)
    dbg = {}
    if _DEBUG:
        dbg["hfin"] = nc.declare_dram_parameter("dbg_hfin", [128, 384], bf16,
                                                isOutput=True)
        dbg["usc"] = nc.declare_dram_parameter("dbg_usc", [128, 1536], bf16,
                                               isOutput=True)
        for r in range(ROUTING + 1):
            dbg[f"rs{r}"] = nc.declare_dram_parameter(
                f"dbg_rs{r}", [128, RC_TOT], bf16, isOutput=True)
            dbg[f"bc{r}"] = nc.declare_dram_parameter(
                f"dbg_bc{r}", [128, 512], bf16, isOutput=True)
            dbg[f"dc{r}"] = nc.declare_dram_parameter(
                f"dbg_dc{r}", [128, 128], bf16, isOutput=True)

    act = nc.scalar
    vec = nc.vector
    pe = nc.tensor
    dma = nc.sync

    with TileContext(nc) as tc:
        with tc.tile_pool(name="const", bufs=1) as cp:
            # ---- load constants ------------------------------------------
            wpx_sb = {}
            for mi, m in enumerate(MODS):
                for ci, (r0, r1) in enumerate(K_CHUNKS[mi]):
                    t = cp.tile([r1 - r0, 512], bf16, name=f"wpx_{m}{ci}")
                    dma.dma_start(out=t, in_=wp["wpx_" + m][r0:r1, :])
                    wpx_sb[(mi, ci)] = t
            wrec_sb = cp.tile([128, 1536], bf16, name="wrec_sb")
            dma.dma_start(out=wrec_sb, in_=wp["wrec"][:, :])
            wcap_sb = cp.tile([128, 1536], bf16, name="wcap_sb")
            dma.dma_start(out=wcap_sb, in_=wp["wcap"][:, :])
            wrih_sb = cp.tile([128, 2048], bf16, name="wrih_sb")
            dma.dma_start(out=wrih_sb, in_=wp["wr_ih"][:, :])
            wrhh_sb = cp.tile([128, 2048], bf16, name="wrhh_sb")
            dma.dma_start(out=wrhh_sb, in_=wp["wr_hh"][:, :])
            wdih_sb = cp.tile([128, 1024], bf16, name="wdih_sb")
            dma.dma_start(out=wdih_sb, in_=wp["wd_ih"][:, :])
            wdhh_sb = cp.tile([128, 1024], bf16, name="wdhh_sb")
            dma.dma_start(out=wdhh_sb, in_=wp["wd_hh"][:, :])
            fc1wt_sb = cp.tile([128, H], bf16, name="fc1wt_sb")
            dma.dma_start(out=fc1wt_sb, in_=wp["fc1wt"][:, :])
            fc2wt_sb = cp.tile([H, 1], bf16, name="fc2wt_sb")
            dma.dma_start(out=fc2wt_sb, in_=wp["fc2wt"][:, :])
            fc1b_sb = cp.tile([H, 1], f32, name="fc1b_sb")
            dma.dma_start(out=fc1b_sb, in_=wp["fc1b"][:, :])
            fc2b_sb = cp.tile([1, 1], f32, name="fc2b_sb")
            dma.dma_start(out=fc2b_sb, in_=wp["fc2b"][:, :])
            ones_sb = cp.tile([128, 128], bf16, name="ones_sb")
            dma.dma_start(out=ones_sb, in_=wp["ones"][:, :])
            rbias_sb = dbias_sb = None
            if has_rbias:
                rbias_sb = cp.tile([1, 2048], bf16, name="rbias_sb")
                dma.dma_start(out=rbias_sb, in_=wp["rb"][:, :])
            if has_dbias:
                dbias_sb = cp.tile([1, 1024], bf16, name="dbias_sb")
                dma.dma_start(out=dbias_sb, in_=wp["db"][:, :])

            # persistent across phases
            usc = cp.tile([128, 1536], bf16, name="usc")

            # ================= ENCODER PHASE ===============================
            if _SKIP_ENC:
                dma.dma_start(out=usc, in_=wp["wcap"][:, :])
            else:
              with tc.tile_pool(name="enc", bufs=4) as ep, \
                 tc.tile_pool(name="encps", bufs=2, space="PSUM") as eps:
                h0 = ep.tile([128, 384], bf16, name="h0", tag="h0", bufs=1)
                dma.dma_start(out=h0, in_=wp["zeros"][:, :])
                h_prev = h0
                c_prev = None
                xw = {}
                hfin = None
                for tau in range(T):
                    w0, j = tau // WIN, tau % WIN
                    if j == 0:
                        for mi in range(3):
                            for ci, (r0, r1) in enumerate(K_CHUNKS[mi]):
                                rows = r1 - r0
                                tf = ep.tile([rows, WIN * BC], bf16,
                                             name=f"xwf{mi}{ci}",
                                             tag=f"xwf{mi}{ci}")
                                dma.dma_start(
                                    out=tf,
                                    in_=xin_d[(mi, ci)][
                                        :, w0 * WIN * BC:(w0 + 1) * WIN * BC])
                                tb = ep.tile([rows, WIN * BC], bf16,
                                             name=f"xwb{mi}{ci}",
                                             tag=f"xwb{mi}{ci}")
                                dma.dma_start(
                                    out=tb,
                                    in_=xin_d[(mi, ci)][
                                        :, (T - (w0 + 1) * WIN) * BC:
                                           (T - w0 * WIN) * BC])
                                xw[("f", mi, ci)] = tf
                                xw[("b", mi, ci)] = tb

                    # ---- matmuls: text uses a 2-step PSUM tile with N=256
                    # px matmuls; audio+video share a 1-step tile.
                    if tau % 2 == 0:
                        gtt = eps.tile([128, 1024], f32, name="gtt",
                                       tag="gtt")
                        for ci in range(len(K_CHUNKS[0])):
                            wq = wpx_sb[(0, ci)]
                            fb2 = xw[("f", 0, ci)][:, j * BC:(j + 2) * BC]
                            bb2 = xw[("b", 0, ci)][
                                :, (WIN - 2 - j) * BC:(WIN - j) * BC
                            ].rearrange("k (s c) -> k s c", c=BC)[:, ::-1, :]
                            for gi in range(4):
                                col = gi * 256
                                st = (ci == 0 and gi in (0, 2))
                                lastt = (_NO_REC and ci == 2 and gi == 3)
                                pe.matmul(
                                    gtt[0:64, col:col + 256],
                                    wq[:, gi * 128:gi * 128 + 64],
                                    fb2, start=st, stop=lastt,
                                    tile_position=(0, 0),
                                    skip_group_check=True)
                                pe.matmul(
                                    gtt[64:128, col:col + 256],
                                    wq[:, gi * 128 + 64:gi * 128 + 128],
                                    bb2, start=st, stop=lastt,
                                    tile_position=(0, 64),
                                    skip_group_check=True)
                        gtt_cur = gtt
                    gav = eps.tile([128, 1024], f32, name="gav", tag="gav")
                    for mi in (1, 2):
                        base = (mi - 1) * 512
                        wq = wpx_sb[(mi, 0)]
                        fb = xw[("f", mi, 0)][:, j * BC:(j + 1) * BC]
                        bb = xw[("b", mi, 0)][
                            :, (WIN - 1 - j) * BC:(WIN - j) * BC]
                        for gi in range(4):
                            col = base + gi * 128
                            st = (gi == 0)
                            lasta = (_NO_REC and gi == 3)
                            pe.matmul(gav[0:64, col:col + 128],
                                      wq[:, gi * 128:gi * 128 + 64],
                                      fb, start=st, stop=lasta,
                                      tile_position=(0, 0),
                                      skip_group_check=True)
                            pe.matmul(gav[64:128, col:col + 128],
                                      wq[:, gi * 128 + 64:gi * 128 + 128],
                                      bb, start=st, stop=lasta,
                                      tile_position=(0, 64),
                                      skip_group_check=True)
                    toff = (tau % 2) * 128
                    for mi in ([] if _NO_REC else (1, 2, 0)):
                        for gi in range(4):
                            if mi == 0:
                                out = gtt_cur[:, gi * 256 + toff:
                                              gi * 256 + toff + 128]
                                stop = (tau % 2 == 1 and gi in (1, 3))
                            else:
                                out = gav[:, (mi - 1) * 512 + gi * 128:
                                          (mi - 1) * 512 + (gi + 1) * 128]
                                stop = (gi == 3)
                            pe.matmul(
                                out,
                                wrec_sb[:, (mi * 4 + gi) * 128:
                                        (mi * 4 + gi + 1) * 128],
                                h_prev[:, mi * 128:(mi + 1) * 128],
                                start=False, stop=stop,
                                skip_group_check=True)

                    sg = ep.tile([128, 1152], bf16, name="sg", tag="sg")
                    tg = ep.tile([128, 384], bf16, name="tg", tag="tg")
                    t1 = ep.tile([128, 384], bf16, name="t1", tag="t1")
                    t2 = ep.tile([128, 384], bf16, name="t2", tag="t2")
                    cc = ep.tile([128, 384], bf16, name="cc", tag="cc")
                    tcn = ep.tile([128, 384], bf16, name="tcn", tag="tcn")
                    hh = ep.tile([128, 384], bf16, name="hh", tag="hh")
                    gt3 = gtt_cur.rearrange("p (g c) -> p g c", c=256)
                    ga3 = gav.rearrange("p (m c) -> p m c", c=512)
                    sgv = sg.rearrange("p (m c) -> p m c", c=384)
                    tg3 = tg.rearrange("p (m c) -> p m c", c=128)
                    t13 = t1.rearrange("p (m c) -> p m c", c=128)
                    t23 = t2.rearrange("p (m c) -> p m c", c=128)
                    cc3 = cc.rearrange("p (m c) -> p m c", c=128)
                    tc3 = tcn.rearrange("p (m c) -> p m c", c=128)
                    hh3 = hh.rearrange("p (m c) -> p m c", c=128)
                    sg128 = sg.rearrange("p (s c) -> p s c", c=128)
                    # sigmoid/tanh per PSUM tensor group (av first: long pole)
                    act.activation(sgv[:, 1:3, :], ga3[:, :, 0:384],
                                   AF.Sigmoid)
                    act.activation(sg128[:, 0:3, :],
                                   gt3[:, 0:3, toff:toff + 128],
                                   AF.Sigmoid)
                    act.activation(tg3[:, 1:3, :], ga3[:, :, 384:512],
                                   AF.Tanh)
                    act.activation(tg3[:, 0:1, :],
                                   gt3[:, 3:4, toff:toff + 128], AF.Tanh)
                    for (m0, m1) in [(0, 1), (1, 3)]:
                        if tau == 0:
                            vec.tensor_tensor(cc3[:, m0:m1, :],
                                              sgv[:, m0:m1, 0:128],
                                              tg3[:, m0:m1, :], MUL)
                        else:
                            cp3 = c_prev.rearrange("p (m c) -> p m c", c=128)
                            vec.tensor_tensor(t23[:, m0:m1, :],
                                              sgv[:, m0:m1, 128:256],
                                              cp3[:, m0:m1, :], MUL)
                            vec.tensor_tensor(t13[:, m0:m1, :],
                                              sgv[:, m0:m1, 0:128],
                                              tg3[:, m0:m1, :], MUL)
                            vec.tensor_tensor(cc3[:, m0:m1, :],
                                              t13[:, m0:m1, :],
                                              t23[:, m0:m1, :], ADD)
                        act.activation(tc3[:, m0:m1, :], cc3[:, m0:m1, :],
                                       AF.Tanh)
                        vec.tensor_tensor(hh3[:, m0:m1, :],
                                          sgv[:, m0:m1, 256:384],
                                          tc3[:, m0:m1, :], MUL)
                    c_prev = cc
                    h_prev = hh
                hfin = h_prev

                # capsule projections: usc[m,k] = Wcap[m,k].T-free matmul
                up1 = eps.tile([128, 1024], f32, name="up1", tag="gtt")
                up2 = eps.tile([128, 512], f32, name="up2", tag="gav")
                for mi in range(3):
                    for k in range(4):
                        col = (mi * 4 + k) * 128
                        outp = up1[:, col:col + 128] if col < 1024 else \
                            up2[:, col - 1024:col - 1024 + 128]
                        pe.matmul(outp,
                                  wcap_sb[:, col:col + 128],
                                  hfin[:, mi * 128:(mi + 1) * 128],
                                  start=True, stop=True)
                vec.tensor_copy(usc[:, 0:1024], up1)
                vec.tensor_copy(usc[:, 1024:1536], up2)
                if _DEBUG:
                    dma.dma_start(out=dbg["hfin"][:, :], in_=hfin)
                    dma.dma_start(out=dbg["usc"][:, :], in_=usc)

            # ================= ROUTING PHASE ===============================
            if not _ENC_ONLY:
              with tc.tile_pool(name="rt", bufs=2) as rp, \
                 tc.tile_pool(name="rtps1", bufs=1, space="PSUM") as rps1, \
                 tc.tile_pool(name="rtps2", bufs=2, space="PSUM") as rps2:
                u12 = usc.rearrange("p (t c) -> p t c", c=128)  # 12 tokens

                rc_soft = None
                dc = None
                for r in range(ROUTING + 1):
                    # ---- softmax(rc) -> rc_soft (bf16) -------------------
                    rs = rp.tile([128, RC_TOT], bf16, name="rs", tag="rs")
                    if r == 0:
                        dma.dma_start(out=rs, in_=wp["rc0"][:, :])
                    else:
                        rcv = rc_carry
                        # exp-based softmax for tav (n=3) and deci (n=7)
                        e3 = rp.tile([128, 384], f32, name="e3", tag="e3")
                        e7 = rp.tile([128, 896], f32, name="e7", tag="e7")
                        act.activation(e3, rcv[:, RC_OFF[3]:RC_OFF[3] + 384],
                                       AF.Exp)
                        act.activation(e7, rcv[:, RC_DECI:RC_TOT], AF.Exp)
                        ssum = rp.tile([128, 256], f32, name="ssum",
                                       tag="ssum")
                        vec.tensor_tensor(ssum[:, 0:128], e3[:, 0:128],
                                          e3[:, 128:256], ADD)
                        vec.tensor_tensor(ssum[:, 0:128], ssum[:, 0:128],
                                          e3[:, 256:384], ADD)
                        vec.tensor_tensor(ssum[:, 128:256], e7[:, 0:128],
                                          e7[:, 128:256], ADD)
                        for jj in range(2, 7):
                            vec.tensor_tensor(ssum[:, 128:256],
                                              ssum[:, 128:256],
                                              e7[:, jj * 128:(jj + 1) * 128],
                                              ADD)
                        rcp = rp.tile([128, 256], f32, name="rcp", tag="rcp")
                        vec.reciprocal(rcp, ssum)
                        for jj in range(3):
                            vec.tensor_tensor(
                                rs[:, RC_OFF[3] + jj * 128:
                                   RC_OFF[3] + (jj + 1) * 128],
                                e3[:, jj * 128:(jj + 1) * 128],
                                rcp[:, 0:128], MUL)
                        for jj in range(7):
                            vec.tensor_tensor(
                                rs[:, RC_DECI + jj * 128:
                                   RC_DECI + (jj + 1) * 128],
                                e7[:, jj * 128:(jj + 1) * 128],
                                rcp[:, 128:256], MUL)
                        # sigmoid-trick softmax for the n=2 chains
                        dfs = rp.tile([128, 384], f32, name="dfs", tag="dfs")
                        for i in range(3):
                            o = RC_OFF[i]
                            vec.tensor_tensor(dfs[:, i * 128:(i + 1) * 128],
                                              rcv[:, o:o + 128],
                                              rcv[:, o + 128:o + 256], SUB)
                        for i in range(3):
                            o = RC_OFF[i]
                            act.activation(rs[:, o:o + 128],
                                           dfs[:, i * 128:(i + 1) * 128],
                                           AF.Sigmoid)
                            act.activation(rs[:, o + 128:o + 256],
                                           dfs[:, i * 128:(i + 1) * 128],
                                           AF.Sigmoid, scale=-1.0)
                    rc_soft = rs
                    if _DEBUG:
                        dma.dma_start(out=dbg[f"rs{r}"][:, :], in_=rs)

                    # ---- xin = rc_soft * pre tokens ----------------------
                    xinp = rp.tile([128, 1152], bf16, name="xinp", tag="xinp")
                    vec.tensor_tensor(xinp[:, 0:256], rc_soft[:, 0:256],
                                      u12[:, 0:8:4, :], MUL)
                    vec.tensor_tensor(xinp[:, 256:512], rc_soft[:, 256:512],
                                      u12[:, 1:9:7, :], MUL)
                    vec.tensor_tensor(xinp[:, 512:768], rc_soft[:, 512:768],
                                      u12[:, 5:10:4, :], MUL)
                    vec.tensor_tensor(xinp[:, 768:1152], rc_soft[:, 768:1152],
                                      u12[:, 2:11:4, :], MUL)
                    xd = rp.tile([128, 896], bf16, name="xd", tag="xd")
                    vec.tensor_tensor(xd[:, 0:384],
                                      rc_soft[:, RC_DECI:RC_DECI + 384],
                                      u12[:, 3:12:4, :], MUL)

                    # ---- pair routing LSTMs ------------------------------
                    bc = rp.tile([128, 512], bf16, name="bc", tag="bc")
                    hp = None
                    cpv = None
                    for s in range(3):
                        chains = [0, 1, 2, 3] if s < 2 else [3]
                        pg = rps1.tile([128, 2048], f32, name="pg",
                                       tag="pair")
                        for i in chains:
                            xtok = xinp[:, (RC_OFF[i] + s * 128):
                                        (RC_OFF[i] + (s + 1) * 128)]
                            for gi in range(4):
                                col = i * 512 + gi * 128
                                pe.matmul(pg[:, col:col + 128],
                                          wrih_sb[:, col:col + 128],
                                          xtok, start=(gi == 0),
                                          stop=(s == 0 and not has_rbias
                                                and gi == 3))
                            if has_rbias:
                                for gi in range(4):
                                    col = i * 512 + gi * 128
                                    pe.matmul(pg[:, col:col + 128],
                                              rbias_sb[0:1, col:col + 128],
                                              ones_sb[0:1, :],
                                              start=False,
                                              stop=(s == 0 and gi == 3))
                            if s > 0:
                                for gi in range(4):
                                    col = i * 512 + gi * 128
                                    pe.matmul(pg[:, col:col + 128],
                                              wrhh_sb[:, col:col + 128],
                                              hp[:, i * 128:(i + 1) * 128],
                                              start=False, stop=(gi == 3))
                        sgp = rp.tile([128, 1536], bf16, name="sgp",
                                      tag="sgp")
                        tgp = rp.tile([128, 512], bf16, name="tgp", tag="tgp")
                        t1p = rp.tile([128, 512], bf16, name="t1p", tag="t1p")
                        t2p = rp.tile([128, 512], f32, name="t2p", tag="t2p")
                        cpn = rp.tile([128, 512], f32, name="cpn", tag="cpn")
                        tcp = rp.tile([128, 512], bf16, name="tcp", tag="tcp")
                        hpn = rp.tile([128, 512], bf16, name="hpn", tag="hpn")
                        pgv = pg.rearrange("p (i c) -> p i c", c=512)
                        sgpv = sgp.rearrange("p (i c) -> p i c", c=384)
                        tgp3 = tgp.rearrange("p (i c) -> p i c", c=128)
                        t1p3 = t1p.rearrange("p (i c) -> p i c", c=128)
                        t2p3 = t2p.rearrange("p (i c) -> p i c", c=128)
                        cpn3 = cpn.rearrange("p (i c) -> p i c", c=128)
                        tcp3 = tcp.rearrange("p (i c) -> p i c", c=128)
                        hpn3 = hpn.rearrange("p (i c) -> p i c", c=128)
                        i0, i1 = (0, 4) if s < 2 else (3, 4)
                        act.activation(sgpv[:, i0:i1, :], pgv[:, i0:i1, 0:384],
                                       AF.Sigmoid)
                        act.activation(tgp3[:, i0:i1, :],
                                       pgv[:, i0:i1, 384:512], AF.Tanh)
                        if s == 0:
                            vec.tensor_tensor(cpn3[:, i0:i1, :],
                                              sgpv[:, i0:i1, 0:128],
                                              tgp3[:, i0:i1, :], MUL)
                        else:
                            cp3v = cpv.rearrange("p (i c) -> p i c", c=128)
                            vec.tensor_tensor(t2p3[:, i0:i1, :],
                                              sgpv[:, i0:i1, 128:256],
                                              cp3v[:, i0:i1, :], MUL)
                            vec.tensor_tensor(t1p3[:, i0:i1, :],
                                              sgpv[:, i0:i1, 0:128],
                                              tgp3[:, i0:i1, :], MUL)
                            vec.tensor_tensor(cpn3[:, i0:i1, :],
                                              t1p3[:, i0:i1, :],
                                              t2p3[:, i0:i1, :], ADD)
                        act.activation(tcp3[:, i0:i1, :], cpn3[:, i0:i1, :],
                                       AF.Tanh)
                        if s == 0:
                            vec.tensor_tensor(hpn3[:, i0:i1, :],
                                              sgpv[:, i0:i1, 256:384],
                                              tcp3[:, i0:i1, :], MUL)
                            hp = hpn
                        elif s == 1:
                            vec.tensor_tensor(bc[:, 0:384],
                                              sgpv[:, 0:3, 256:384],
                                              tcp3[:, 0:3, :], MUL)
                            vec.tensor_tensor(hpn3[:, 3:4, :],
                                              sgpv[:, 3:4, 256:384],
                                              tcp3[:, 3:4, :], MUL)
                            hp = hpn
                        else:
                            vec.tensor_tensor(bc[:, 384:512],
                                              sgpv[:, 3:4, 256:384],
                                              tcp3[:, 3:4, :], MUL)
                        cpv = cpn

                    # ---- decision biLSTM input: xd bc part ---------------
                    vec.tensor_tensor(xd[:, 384:896],
                                      rc_soft[:, RC_DECI + 384:RC_TOT],
                                      bc, MUL)

                    # ---- decision biLSTM ---------------------------------
                    hd = None
                    cdv = None
                    for s in range(7):
                        pd = rps2.tile([128, 1024], f32, name="pd",
                                       tag="deci")
                        for di in range(2):
                            tok = s if di == 0 else 6 - s
                            xtok = xd[:, tok * 128:(tok + 1) * 128]
                            for gi in range(4):
                                col = di * 512 + gi * 128
                                pe.matmul(pd[:, col:col + 128],
                                          wdih_sb[:, col:col + 128],
                                          xtok, start=(gi == 0),
                                          stop=(s == 0 and not has_dbias
                                                and gi == 3))
                            if has_dbias:
                                for gi in range(4):
                                    col = di * 512 + gi * 128
                                    pe.matmul(pd[:, col:col + 128],
                                              dbias_sb[0:1, col:col + 128],
                                              ones_sb[0:1, :],
                                              start=False,
                                              stop=(s == 0 and gi == 3))
                            if s > 0:
                                for gi in range(4):
                                    col = di * 512 + gi * 128
                                    pe.matmul(pd[:, col:col + 128],
                                              wdhh_sb[:, col:col + 128],
                                              hd[:, di * 128:(di + 1) * 128],
                                              start=False, stop=(gi == 3))
                        sgd = rp.tile([128, 768], bf16, name="sgd", tag="sgd")
                        tgd = rp.tile([128, 256], bf16, name="tgd", tag="tgd")
                        t1d = rp.tile([128, 256], bf16, name="t1d", tag="t1d")
                        t2d = rp.tile([128, 256], f32, name="t2d", tag="t2d")
                        cdn = rp.tile([128, 256], f32, name="cdn", tag="cdn")
                        tcd = rp.tile([128, 256], bf16, name="tcd", tag="tcd")
                        hdn = rp.tile([128, 256], bf16, name="hdn", tag="hdn")
                        pdv = pd.rearrange("p (i c) -> p i c", c=512)
                        sgdv = sgd.rearrange("p (i c) -> p i c", c=384)
                        tgd3 = tgd.rearrange("p (i c) -> p i c", c=128)
                        t1d3 = t1d.rearrange("p (i c) -> p i c", c=128)
                        t2d3 = t2d.rearrange("p (i c) -> p i c", c=128)
                        cdn3 = cdn.rearrange("p (i c) -> p i c", c=128)
                        tcd3 = tcd.rearrange("p (i c) -> p i c", c=128)
                        hdn3 = hdn.rearrange("p (i c) -> p i c", c=128)
                        act.activation(sgdv[:, :, :], pdv[:, :, 0:384],
                                       AF.Sigmoid)
                        act.activation(tgd3[:, :, :], pdv[:, :, 384:512],
                                       AF.Tanh)
                        if s == 0:
                            vec.tensor_tensor(cdn3[:, :, :], sgdv[:, :, 0:128],
                                              tgd3[:, :, :], MUL)
                        else:
                            cd3 = cdv.rearrange("p (i c) -> p i c", c=128)
                            vec.tensor_tensor(t2d3[:, :, :],
                                              sgdv[:, :, 128:256],
                                              cd3[:, :, :], MUL)
                            vec.tensor_tensor(t1d3[:, :, :],
                                              sgdv[:, :, 0:128],
                                              tgd3[:, :, :], MUL)
                            vec.tensor_tensor(cdn3[:, :, :], t1d3[:, :, :],
                                              t2d3[:, :, :], ADD)
                        act.activation(tcd3[:, :, :], cdn3[:, :, :], AF.Tanh)
                        vec.tensor_tensor(hdn3[:, :, :], sgdv[:, :, 256:384],
                                          tcd3[:, :, :], MUL)
                        hd = hdn
                        cdv = cdn
                    dc = rp.tile([128, 128], bf16, name="dc", tag="dc")
                    vec.tensor_tensor(dc, hd[:, 0:128], hd[:, 128:256], ADD)
                    if _DEBUG:
                        dma.dma_start(out=dbg[f"bc{r}"][:, :], in_=bc)
                        dma.dma_start(out=dbg[f"dc{r}"][:, :], in_=dc)

                    # ---- agreement update (r < ROUTING) ------------------
                    if r < ROUTING:
                        tp = rp.tile([128, 2048], bf16, name="tp", tag="tp")
                        off = 0
                        for i in range(4):
                            for (mi, k) in PAIR_TOKENS[i]:
                                vec.tensor_tensor(
                                    tp[:, off:off + 128],
                                    usc[:, (mi * 4 + k) * 128:
                                        (mi * 4 + k + 1) * 128],
                                    bc[:, i * 128:(i + 1) * 128], MUL)
                                off += 128
                        for (mi, k) in DECI_USC_TOKENS:
                            vec.tensor_tensor(
                                tp[:, off:off + 128],
                                usc[:, (mi * 4 + k) * 128:
                                    (mi * 4 + k + 1) * 128],
                                dc, MUL)
                            off += 128
                        for jj in range(4):
                            vec.tensor_tensor(
                                tp[:, off:off + 128],
                                bc[:, jj * 128:(jj + 1) * 128], dc, MUL)
                            off += 128
                        agp = rps1.tile([128, 2048], f32, name="agp",
                                        tag="pair")
                        for c0 in range(0, 2048, 512):
                            pe.matmul(agp[:, c0:c0 + 512], ones_sb,
                                      tp[:, c0:c0 + 512],
                                      start=True, stop=True)
                        rc_carry = rp.tile([128, RC_TOT], f32, name="rcc",
                                           tag="rcc")
                        vec.tensor_tensor(rc_carry, rc_soft, agp, ADD)

                # ---- fc head ----------------------------------------------
                fcp = rps2.tile([H, 128], f32, name="fcp", tag="deci")
                pe.matmul(fcp, fc1wt_sb, dc, start=True, stop=True)
                o1 = rp.tile([H, 128], bf16, name="o1", tag="o1")
                act.activation(o1, fcp, AF.Tanh, bias=fc1b_sb)
                fcp2 = rps2.tile([1, 128], f32, name="fcp2", tag="deci")
                pe.matmul(fcp2, fc2wt_sb, o1, start=True, stop=True)
                out_sb = rp.tile([1, 128], f32, name="out_sb", tag="out_sb")
                act.activation(out_sb, fcp2, AF.Identity, bias=fc2b_sb)
                dma.dma_start(out=out_d[:, :], in_=out_sb)

    nc.finalize()
    return nc


# ---------------------------------------------------------------------------
# driver
# ---------------------------------------------------------------------------

_PROGRAM_CACHE = {}


def _kernel_bass(inputs):
    import ml_dtypes
    from concourse.bass_utils import run_bass_kernel_spmd

    bf16 = ml_dtypes.bfloat16
    w = {k: np.asarray(inputs[k], np.float32) for k in _WEIGHT_KEYS}
    wd = _pack_weights(w, bf16)
    has_rbias = bool(np.any(w["r_b"] != 0.0))
    has_dbias = bool(np.any(w["d_b_f"] != 0.0) or np.any(w["d_b_b"] != 0.0))

    key = (has_rbias, has_dbias)
    if key not in _PROGRAM_CACHE:
        _PROGRAM_CACHE[key] = _build_program(has_rbias, has_dbias)
    nc = _PROGRAM_CACHE[key]

    xt = _pack_x(np.asarray(inputs["text"], np.float32), T_IN, bf16)
    xa = _pack_x(np.asarray(inputs["audio"], np.float32), A_IN, bf16)
    xv = _pack_x(np.asarray(inputs["video"], np.float32), V_IN, bf16)
    xs = [xt, xa, xv]

    in_maps = []
    for c in range(N_CORES):
        m = {}
        for mi, mn in enumerate(MODS):
            for ci, (r0, r1) in enumerate(K_CHUNKS[mi]):
                m[f"x_{mn}{ci}"] = np.ascontiguousarray(xs[mi][c][r0:r1])
        for k in ("wpx_t", "wpx_a", "wpx_v", "wrec", "wcap", "wr_ih", "wr_hh",
                  "wd_ih", "wd_hh", "rb", "db", "fc1wt", "fc2wt", "fc1b",
                  "fc2b", "ones", "zeros", "rc0"):
            m[k] = wd[k]
        in_maps.append(m)

    res = run_bass_kernel_spmd(nc, in_maps, list(range(N_CORES)))
    outs = [np.asarray(res.results[c]["out"], np.float32).reshape(BC)
            for c in range(N_CORES)]
    out = np.concatenate(outs).reshape(B, 1)
    if not np.all(np.isfinite(out)):
        raise RuntimeError("non-finite output from bass kernel")
    return out


# ---------------------------------------------------------------------------
# numpy fallback (reference math)
# ---------------------------------------------------------------------------

def _forward_numpy(text, audio, video, w):
    def sigmoid(x):
        return 1.0 / (1.0 + np.exp(-x))

    def lstm_final(x, Wih, Whh, b):
        Bs = x.shape[0]
        Hh = Whh.shape[-1]
        h = np.zeros((Bs, Hh), np.float32)
        c = np.zeros((Bs, Hh), np.float32)
        px = np.einsum('btd,gd->btg', x, Wih, optimize=True) + b
        for t in range(x.shape[1]):
            g = px[:, t] + h @ Whh.T
            i, f, gg, o = np.split(g, 4, axis=-1)
            c = sigmoid(f) * c + sigmoid(i) * np.tanh(gg)
            h = sigmoid(o) * np.tanh(c)
        return h

    def ctx(x, Wf, Uf, bf, Wb, Ub, bb):
        hf = lstm_final(x, Wf, Uf, bf)
        hb = lstm_final(x[:, ::-1], Wb, Ub, bb)
        return np.concatenate([hf, hb], -1)[:, None, :]

    def softmax(x, axis):
        m = x.max(axis=axis, keepdims=True)
        e = np.exp(x - m)
        return e / e.sum(axis=axis, keepdims=True)

    Bsz = text.shape[0]
    tc = ctx(text, w["t_Wih_f"], w["t_Whh_f"], w["t_b_f"],
             w["t_Wih_b"], w["t_Whh_b"], w["t_b_b"])
    ac = ctx(audio, w["a_Wih_f"], w["a_Whh_f"], w["a_b_f"],
             w["a_Wih_b"], w["a_Whh_b"], w["a_b_b"])
    vc = ctx(video, w["v_Wih_f"], w["v_Whh_f"], w["v_b_f"],
             w["v_Wih_b"], w["v_Whh_b"], w["v_b_b"])

    tusc = np.einsum('bod,kde->kboe', tc, w["Wt"])
    ausc = np.einsum('bod,kde->kboe', ac, w["Wa"])
    vusc = np.einsum('bod,kde->kboe', vc, w["Wv"])

    pre = [np.concatenate([tusc[0], ausc[0]], 1),
           np.concatenate([tusc[1], vusc[0]], 1),
           np.concatenate([ausc[1], vusc[1]], 1),
           np.concatenate([tusc[2], ausc[2], vusc[2]], 1)]

    rc = [np.ones((Bsz, n, D), np.float32) for n in (2, 2, 2, 3, 7)]
    dc = None
    for r in range(ROUTING + 1):
        rc = [softmax(c, 1) for c in rc]
        bcl = [lstm_final(rc[i] * pre[i], w["r_Wih"][i], w["r_Whh"][i],
                          w["r_b"][i])[:, None, :] for i in range(4)]
        deci = np.concatenate([tusc[3], ausc[3], vusc[3]] + bcl, 1)
        xdv = rc[4] * deci
        dc = (lstm_final(xdv, w["d_Wih_f"], w["d_Whh_f"], w["d_b_f"])
              + lstm_final(xdv[:, ::-1], w["d_Wih_b"], w["d_Whh_b"],
                           w["d_b_b"]))[:, None, :]
        if r < ROUTING:
            rc = [rc[i] + np.matmul(pre[i], np.swapaxes(bcl[i], 1, 2))
                  for i in range(4)] \
                 + [rc[4] + np.matmul(deci, np.swapaxes(dc, 1, 2))]

    dc = dc[:, 0, :]
    o1 = np.tanh(dc @ w["fc1_W"].T + w["fc1_b"])
    return o1 @ w["fc2_W"].T + w["fc2_b"]


def kernel(**inputs):
    try:
        return _kernel_bass(inputs)
    except Exception:
        traceback.print_exc()
    text = np.asarray(inputs["text"], np.float32)
    audio = np.asarray(inputs["audio"], np.float32)
    video = np.asarray(inputs["video"], np.float32)
    w = {k: np.asarray(inputs[k], np.float32) for k in _WEIGHT_KEYS}
    return _forward_numpy(text, audio, video, w).astype(np.float32)

